# revision 18
# baseline (speedup 1.0000x reference)
"""Distributed causal multi-head attention for 8 TRN2 NeuronCores.

Problem: B=2, T=2048, D=1024, H=16 heads (hd=64), f32 in/out.

Sharding: core i handles batch b=i//4 and head-group g=i%4 (4 heads).
Wq/Wk/Wv column-sharded ([1024, 256] per core), Wo row-sharded
([256, 1024] per core).  Each core computes a partial output projection
for its 4 heads over the full sequence; the host sums the 4 partials
per batch (the unshard step replaces the all-reduce).  As part of
sharding, the host pre-casts weights/activations to bf16 (the kernel's
compute dtype) and lays x out transposed (xT = x^T), so the device
spends no cycles on input formatting.

Per-core dataflow (matmuls bf16 on TensorEngine, f32 accumulation):
  QT,KT [256(d),2048(t)] = W^T @ x^T   (d on partitions)
  V     [2048(t),256(d)]               (t on partitions, +ones col)
  ST[k,q] = K . Q^T  -> exp (ACT, scale=1/sqrt(64)) -> PT bf16
  causal: diagonal tiles narrowed to their valid q range; only the
  128-wide diagonal block needs masking (Pool multiply by a tri mask)
  AV: out[q, 65] += PT[k,q]^T @ Vaug[k, 65]  (col 64 = softmax denom)
  attn[q, dv] = AV[:, :64] * recip(AV[:, 64])  (DVE per-partition)
  attnT via PE transpose -> out_partial[t,e] = attnT^T @ Wo

Schedule: inputs arrive via consolidated multi-dim DMAs on 4 engine
rings, xT in column-stripe order so the V projection (stationary = x
stripes) starts after ~1MB.  Scores for (slab 0, head 0) begin after
only 4 projection chunks; the remaining projection work is injected as
PE filler into the attention stream.  Pair order puts a tiny slab-0
pair last so the drain chain at the end is short; slab epilogues are
deferred into the ACT-heavy slab-3 stretch to keep PE busy there.
PSUM->SBUF casts/copies run on DVE (Pool cannot touch PSUM); the
diagonal causal masks run on Pool so the DVE queue stays short.
"""

import numpy as np
import ml_dtypes

import concourse.bass as bass
import concourse.mybir as mybir
import concourse.tile as tile
from concourse import bacc
from concourse.bass_utils import run_bass_kernel_spmd
from concourse.masks import make_identity

F32 = mybir.dt.float32
BF16 = mybir.dt.bfloat16
AF = mybir.ActivationFunctionType

T = 2048  # sequence length
D = 1024  # embed dim
NH = 4  # heads per core
HD = 64  # head dim
DH = NH * HD  # 256, sharded d per core
TT = T // 128  # 16 t tiles
DT = D // 128  # 8 embed tiles
NSLAB = 4  # q slabs of 512
SCALE = 1.0 / np.sqrt(HD)

_NC_CACHE = None

def build():
    nc = bacc.Bacc(None, target_bir_lowering=False, debug=False)

    # host pre-layouts: partition-major so every DMA descriptor is one
    # contiguous 4KB run per partition
    xT_ext = nc.declare_dram_parameter("xT", [128, 8 * DT * 256], BF16, isOutput=False)
    wq = nc.declare_dram_parameter("Wq", [128, DT * DH], BF16, isOutput=False)
    wk = nc.declare_dram_parameter("Wk", [128, DT * DH], BF16, isOutput=False)
    wv = nc.declare_dram_parameter("Wv", [128, DT * DH], BF16, isOutput=False)
    wo = nc.declare_dram_parameter("Wo", [128, 2 * D], BF16, isOutput=False)
    out = nc.declare_dram_parameter("out", [T, D], F32, isOutput=True)

    with tile.TileContext(nc) as tc:
        with (
            tc.tile_pool(name="persist", bufs=1) as persist,
            tc.tile_pool(name="pt", bufs=2) as pt_pool,
            tc.tile_pool(name="opev", bufs=3) as opev_pool,
            tc.tile_pool(name="avstg", bufs=2) as avstg_pool,
            tc.tile_pool(name="recip", bufs=4) as recip_pool,
            tc.tile_pool(name="ps_st", bufs=3, space="PSUM") as ps_st,
            tc.tile_pool(name="ps_av", bufs=2, space="PSUM") as ps_av,
        ):
            def P(shape, dtype, name):
                return persist.tile(shape, dtype, name=name, tag=name)

            ident_b = P([128, 128], BF16, "ident_b")
            make_identity(nc, ident_b)

            wq_bf = P([128, DT * DH], BF16, "wq_bf")
            wk_bf = P([128, DT * DH], BF16, "wk_bf")
            wv_bf = P([128, DT * DH], BF16, "wv_bf")
            wo_bf = P([128, 2 * D], BF16, "wo_bf")
            xT = P([128, DT * T], BF16, "xT")
            QT = P([128, 2 * T], BF16, "QT")
            KT = P([128, 2 * T], BF16, "KT")
            vbuf = P([128, TT * NH * 65], BF16, "vbuf")
            attn = P([128, TT * DH], BF16, "attn")
            attnT = P([128, 2 * T], BF16, "attnT")

            # ---- input DMAs: host layout is partition-major, so each
            # dma_start is 128 descriptors of one contiguous 4KB run.
            # xT arrives in 256-col stripes (all 8 dt chunks per stripe)
            # so V tiles and QK chunks unlock incrementally. ----
            SW = 8 * 256  # sbuf cols per stripe (8 dt x 256 t)

            def x_stripe(eng, j):
                eng.dma_start(
                    out=xT[:, j * SW : (j + 1) * SW],
                    in_=xT_ext[:, j * SW : (j + 1) * SW],
                )

            nc.scalar.dma_start(out=wv_bf[:], in_=wv[:])
            x_stripe(nc.sync, 0)
            nc.scalar.dma_start(out=wq_bf[:], in_=wq[:])
            x_stripe(nc.sync, 1)
            nc.scalar.dma_start(out=wk_bf[:], in_=wk[:])
            x_stripe(nc.sync, 2)
            nc.scalar.dma_start(out=wo_bf[:], in_=wo[:])
            for j in range(3, 8):
                x_stripe(nc.sync, j)

            vb3 = vbuf.rearrange("p (t c) -> p t c", c=65)
            nc.gpsimd.memset(vb3[:, :, 64:65], 1.0)

            # causal mask for the 128-wide diagonal blocks: keep col >= row
            tri = P([128, 128], BF16, "tri")
            nc.gpsimd.memset(tri[:], 1.0)
            nc.gpsimd.affine_select(
                out=tri[:],
                in_=tri[:],
                pattern=[[1, 128]],
                compare_op=mybir.AluOpType.is_ge,
                fill=0.0,
                base=0,
                channel_multiplier=-1,
            )

            xT4 = xT.rearrange("p (j dt c) -> p j dt c", j=8, dt=DT)

            def qk_chunk(w_bf, outT, m, c):
                """One 512-col QK projection chunk (PE + DVE cast)."""
                def go():
                    ps = ps_st.tile([128, 512], F32, name="psst", tag="ps")
                    for dt_ in range(DT):
                        nc.tensor.matmul(
                            ps[:],
                            lhsT=w_bf[
                                :, dt_ * DH + m * 128 : dt_ * DH + (m + 1) * 128
                            ],
                            rhs=xT4[:, 2 * c : 2 * c + 2, dt_, :],
                            start=(dt_ == 0),
                            stop=(dt_ == DT - 1),
                        )
                    nc.vector.tensor_copy(
                        outT[:, m * T + c * 512 : m * T + (c + 1) * 512], ps[:]
                    )

                return go

            vb4 = vbuf.rearrange("p (n c) -> p n c", c=65)

            def v_chunk(tt):
                """One V-projection t-tile (PE + Pool cast)."""
                def go():
                    ps = ps_st.tile([128, 512], F32, name="psst", tag="ps")
                    j, o = tt // 2, (tt % 2) * 128
                    for dt_ in range(DT):
                        nc.tensor.matmul(
                            ps[:, 0:256],
                            lhsT=xT4[:, j, dt_, o : o + 128],
                            rhs=wv_bf[:, dt_ * DH : (dt_ + 1) * DH],
                            start=(dt_ == 0),
                            stop=(dt_ == DT - 1),
                        )
                    nc.vector.tensor_copy(
                        vb4[:, tt * NH : (tt + 1) * NH, 0:64],
                        ps[:, 0:256].rearrange("p (n c) -> p n c", n=NH),
                    )

                return go

            def pt_layout(s):
                """Compact per-pair PT layout: col base and q-offset per kt."""
                base, off, b = {}, {}, 0
                for kt in range(4 * (s + 1)):
                    j = kt - 4 * s
                    o = 128 * j if j > 0 else 0
                    base[kt], off[kt] = b, o
                    b += 512 - o
                return base, off

            def scores_chunks(s, h, pt):
                m, r0 = h // 2, (h % 2) * 64
                base, _ = pt_layout(s)

                def off_diag(kt):
                    def go():
                        ps = ps_st.tile([128, 1024], F32, name="psst", tag="ps")
                        for u in range(2):
                            nc.tensor.matmul(
                                ps[:, u * 512 : (u + 1) * 512],
                                lhsT=KT[
                                    r0 : r0 + 64,
                                    m * T + (kt + u) * 128 : m * T + (kt + u + 1) * 128,
                                ],
                                rhs=QT[
                                    r0 : r0 + 64,
                                    m * T + s * 512 : m * T + (s + 1) * 512,
                                ],
                                start=True,
                                stop=True,
                            )
                        nc.scalar.activation(
                            out=pt[:, base[kt] : base[kt] + 1024],
                            in_=ps[:],
                            func=AF.Exp,
                            scale=float(SCALE),
                        )

                    return go

                def diag2(j0):
                    # two diagonal tiles (j0, j0+1) packed into one psum/exp
                    widths = [512 - 128 * j0, 512 - 128 * (j0 + 1)]
                    wtot = sum(widths)

                    def go():
                        ps = ps_st.tile([128, 1024], F32, name="psst", tag="ps")
                        o = 0
                        for u, w in enumerate(widths):
                            j = j0 + u
                            kt = 4 * s + j
                            nc.tensor.matmul(
                                ps[:, o : o + w],
                                lhsT=KT[
                                    r0 : r0 + 64,
                                    m * T + kt * 128 : m * T + (kt + 1) * 128,
                                ],
                                rhs=QT[
                                    r0 : r0 + 64,
                                    m * T + s * 512 + 128 * j : m * T + (s + 1) * 512,
                                ],
                                start=True,
                                stop=True,
                            )
                            o += w
                        kt0 = 4 * s + j0
                        nc.scalar.activation(
                            out=pt[:, base[kt0] : base[kt0] + wtot],
                            in_=ps[:, 0:wtot],
                            func=AF.Exp,
                            scale=float(SCALE),
                        )
                        for u in range(2):
                            kt = 4 * s + j0 + u
                            nc.gpsimd.tensor_mul(
                                pt[:, base[kt] : base[kt] + 128],
                                pt[:, base[kt] : base[kt] + 128],
                                tri[:],
                            )

                    return go

                return [off_diag(2 * u) for u in range(2 * s)] + [diag2(0), diag2(2)]

            def av_ops(s, h, pt, eager=False):
                """V-stationary AV accumulation; batched transpose+norm.
                eager=True computes recip/norm per q-tile immediately so the
                drain epilogue can start while later tiles still transpose."""
                base, off = pt_layout(s)
                nk = 4 * (s + 1)
                stg = {}

                def av_go():
                    avb = ps_av.tile([128, 512], F32, name="psav", tag="psav")
                    for kt in range(nk):
                        o = off[kt]
                        nc.tensor.matmul(
                            avb[0:65, o:512],
                            lhsT=vb4[:, kt * NH + h, :],
                            rhs=pt[:, base[kt] : base[kt] + 512 - o],
                            start=(kt == 0),
                            stop=(kt == nk - 1),
                        )
                    st = avstg_pool.tile([65, 512], BF16, name="avst")
                    stg["st"] = st
                    nc.vector.tensor_copy(st[:], avb[0:65, :])

                pnst = {}

                def tr_go(qi):
                    def go():
                        st = stg["st"]
                        if qi == 0:
                            pnst["pn"] = ps_av.tile(
                                [128, 264], BF16, name="psn", tag="psav"
                            )
                            pnst["rc"] = recip_pool.tile(
                                [128, 4], F32, name="rc"
                            )
                        pn = pnst["pn"]
                        nc.tensor.transpose(
                            pn[:, qi * 66 : qi * 66 + 65],
                            st[:, qi * 128 : (qi + 1) * 128],
                            ident_b[0:65, 0:65],
                        )
                        rc = pnst["rc"]
                        pn66 = pn.rearrange("p (n c) -> p n c", c=66)
                        if eager:
                            nc.vector.reciprocal(
                                rc[:, qi : qi + 1], pn66[:, qi : qi + 1, 64]
                            )
                        elif qi == 3:
                            nc.vector.reciprocal(rc[:], pn66[:, :, 64])

                    return go

                def norm_go(qi):
                    def go():
                        qt = 4 * s + qi
                        pn, rc = pnst["pn"], pnst["rc"]
                        nc.vector.tensor_scalar_mul(
                            attn[:, qt * DH + h * 64 : qt * DH + (h + 1) * 64],
                            pn[:, qi * 66 : qi * 66 + 64],
                            rc[:, qi : qi + 1],
                        )

                    return go

                if eager:
                    ops = [av_go]
                    for qi in range(4):
                        ops += [tr_go(qi), norm_go(qi)]
                    return ops
                return (
                    [av_go]
                    + [tr_go(qi) for qi in range(4)]
                    + [norm_go(qi) for qi in range(4)]
                )

            at3 = attnT.rearrange("p (i t) -> p i t", i=2)

            def epilogue_ops(s):
                ops = []
                for qt in range(4 * s, 4 * (s + 1)):
                    def tr(qt=qt):
                        ps = ps_av.tile([128, 256], BF16, name="pstrb", tag="psav")
                        for i in range(2):
                            nc.tensor.transpose(
                                ps[:, i * 128 : (i + 1) * 128],
                                attn[:, qt * DH + i * 128 : qt * DH + (i + 1) * 128],
                                ident_b[:],
                            )
                        nc.vector.tensor_copy(
                            at3[:, :, qt * 128 : (qt + 1) * 128],
                            ps.rearrange("p (i c) -> p i c", i=2),
                        )

                    ops.append(tr)
                for tt in range(4 * s, 4 * (s + 1)):
                    def op_(tt=tt):
                        ps = ps_st.tile([128, 1024], F32, name="psst", tag="ps")
                        for i in range(2):
                            lhsT = attnT[:, i * T + tt * 128 : i * T + (tt + 1) * 128]
                            for ec in range(2):
                                nc.tensor.matmul(
                                    ps[:, ec * 512 : (ec + 1) * 512],
                                    lhsT=lhsT,
                                    rhs=wo_bf[
                                        :, i * D + ec * 512 : i * D + (ec + 1) * 512
                                    ],
                                    start=(i == 0),
                                    stop=(i == 1),
                                )
                        ev = opev_pool.tile([128, 1024], F32, name="ev")
                        nc.vector.tensor_copy(ev[:], ps[:])
                        nc.sync.dma_start(
                            out=out[tt * 128 : (tt + 1) * 128, :], in_=ev[:]
                        )

                    ops.append(op_)
                return ops

            def interleave(a, b):
                if not a:
                    return list(b)
                if not b:
                    return list(a)
                res = []
                nb, na, bi = len(b), len(a), 0
                for i, op in enumerate(a):
                    res.append(op)
                    want = (i + 1) * nb // na
                    while bi < want:
                        res.append(b[bi])
                        bi += 1
                res.extend(b[bi:])
                return res

            # projection thunks: qk[(w, m, c)] and v[tt]
            qkt = {}
            for w_bf, outT, wn in ((wq_bf, QT, "q"), (wk_bf, KT, "k")):
                for m in range(2):
                    for c in range(4):
                        qkt[(wn, m, c)] = qk_chunk(w_bf, outT, m, c)
            vt = {tt: v_chunk(tt) for tt in range(TT)}

            # ---- minimal prologue: just enough for (slab 0, head 0) ----
            for op in (vt[0], vt[1], qkt[("q", 0, 0)], qkt[("k", 0, 0)]):
                op()

            # remaining projection work as PE filler, placed just before
            # its first consumer in the pair stream
            fill_at = {
                0: [vt[2], vt[3], qkt[("q", 1, 0)], qkt[("k", 1, 0)]],
                1: [qkt[("q", 0, 1)], qkt[("k", 0, 1)]],
                2: [vt[4], vt[5]],
                3: [vt[6], vt[7], qkt[("q", 1, 1)]],
                4: [qkt[("k", 1, 1)], qkt[("q", 0, 2)]],
                5: [qkt[("k", 0, 2)], vt[8]],
                6: [vt[9], vt[10], vt[11]],
                7: [qkt[("q", 1, 2)], qkt[("k", 1, 2)]],
                8: [qkt[("q", 0, 3)], qkt[("k", 0, 3)]],
                9: [qkt[("q", 1, 3)], qkt[("k", 1, 3)]],
                10: [vt[12]],
                11: [vt[13]],
                12: [vt[14], vt[15]],
            }
            # slab epilogues: deferred into the ACT-heavy slab-3 stretch
            epi_at = {13: 1, 14: 2, 15: 3}

            pairs = (
                [(0, 0), (0, 1), (0, 2)]
                + [(1, h) for h in range(4)]
                + [(2, h) for h in range(4)]
                + [(3, h) for h in range(4)]
                + [(0, 3)]
            )
            pts = {}
            prev = None
            for idx in range(len(pairs) + 1):
                sc = []
                if idx < len(pairs):
                    s, h = pairs[idx]
                    pts[idx] = pt_pool.tile([128, TT * 512], BF16, name="pt")
                    sc = scores_chunks(s, h, pts[idx])
                av = []
                if prev is not None:
                    ps_, ph_ = pairs[prev]
                    av = av_ops(ps_, ph_, pts[prev], eager=(idx == len(pairs)))
                fill = fill_at.get(idx, [])
                epi = epilogue_ops(epi_at[idx]) if idx in epi_at else []
                if idx == len(pairs):
                    # drain: eager AV of (0,3) interleaved with the slab-0
                    # epilogue so PE never waits on the norm chain
                    e0 = epilogue_ops(0)
                    order = (
                        av[0:5]
                        + [e0[0], av[5], av[6], e0[1], av[7], av[8]]
                        + e0[2:]
                    )
                    for op in order:
                        op()
                else:
                    for op in interleave(sc, fill + av + epi):
                        op()
                prev = idx

    nc.compile()
    return nc


def _get_nc():
    global _NC_CACHE
    if _NC_CACHE is None:
        _NC_CACHE = build()
    return _NC_CACHE


def _pmajor_w(w):
    # [1024, 256] -> [128, 8*256]: whost[p, dt*256+c] = w[dt*128+p, c]
    return np.ascontiguousarray(
        w.reshape(DT, 128, DH).transpose(1, 0, 2).reshape(128, DT * DH)
    )


def make_in_maps(x, Wq, Wk, Wv, Wo):
    bf = ml_dtypes.bfloat16
    x = np.asarray(x, dtype=np.float32)
    WqT = np.asarray(Wq, dtype=np.float32).astype(bf)
    WkT = np.asarray(Wk, dtype=np.float32).astype(bf)
    WvT = np.asarray(Wv, dtype=np.float32).astype(bf)
    WoT = np.asarray(Wo, dtype=np.float32).astype(bf)
    # xT host layout: [p, j, dt, c] (j = 256-col stripe, dt = 128-row chunk)
    xTb = []
    for b in range(2):
        xt = x[b].T.astype(bf)  # [1024, 2048]
        xt = xt.reshape(DT, 128, 8, 256).transpose(1, 2, 0, 3)
        xTb.append(np.ascontiguousarray(xt.reshape(128, 8 * DT * 256)))
    in_maps = []
    for core in range(8):
        b, g = core // 4, core % 4
        sl = slice(g * DH, (g + 1) * DH)
        wo_c = WoT[sl, :]  # [256, 1024]
        wo_p = np.ascontiguousarray(
            wo_c.reshape(2, 128, D).transpose(1, 0, 2).reshape(128, 2 * D)
        )
        in_maps.append(
            {
                "xT": xTb[b],
                "Wq": _pmajor_w(WqT[:, sl]),
                "Wk": _pmajor_w(WkT[:, sl]),
                "Wv": _pmajor_w(WvT[:, sl]),
                "Wo": wo_p,
            }
        )
    return in_maps


def unshard(results):
    out = np.empty((2, T, D), np.float32)
    for b in range(2):
        out[b] = results[4 * b]["out"]
        for g in range(1, 4):
            out[b] += results[4 * b + g]["out"]
    return out


def kernel(x, Wq, Wk, Wv, Wo):
    nc = _get_nc()
    in_maps = make_in_maps(x, Wq, Wk, Wv, Wo)
    res = run_bass_kernel_spmd(nc, in_maps, core_ids=list(range(8)))
    return unshard(res.results)


# revision 19
# speedup vs baseline: 1.0143x; 1.0143x over previous
"""Distributed causal multi-head attention for 8 TRN2 NeuronCores.

Problem: B=2, T=2048, D=1024, H=16 heads (hd=64), f32 in/out.

Sharding: core i handles batch b=i//4 and head-group g=i%4 (4 heads).
Wq/Wk/Wv column-sharded ([1024, 256] per core), Wo row-sharded
([256, 1024] per core).  Each core computes a partial output projection
for its 4 heads over the full sequence; the host sums the 4 partials
per batch (the unshard step replaces the all-reduce).  As part of
sharding, the host pre-casts weights/activations to bf16 (the kernel's
compute dtype) and lays x out transposed (xT = x^T), so the device
spends no cycles on input formatting.

Per-core dataflow (matmuls bf16 on TensorEngine, f32 accumulation):
  QT,KT [256(d),2048(t)] = W^T @ x^T   (d on partitions)
  V     [2048(t),256(d)]               (t on partitions, +ones col)
  ST[k,q] = K . Q^T  -> exp (ACT, scale=1/sqrt(64)) -> PT bf16
  causal: diagonal tiles narrowed to their valid q range; only the
  128-wide diagonal block needs masking (Pool multiply by a tri mask)
  AV: out[q, 65] += PT[k,q]^T @ Vaug[k, 65]  (col 64 = softmax denom)
  attn[q, dv] = AV[:, :64] * recip(AV[:, 64])  (DVE per-partition)
  attnT via PE transpose -> out_partial[t,e] = attnT^T @ Wo

Schedule: inputs arrive via consolidated multi-dim DMAs on 4 engine
rings, xT in column-stripe order so the V projection (stationary = x
stripes) starts after ~1MB.  Scores for (slab 0, head 0) begin after
only 4 projection chunks; the remaining projection work is injected as
PE filler into the attention stream.  Pair order puts a tiny slab-0
pair last so the drain chain at the end is short; slab epilogues are
deferred into the ACT-heavy slab-3 stretch to keep PE busy there.
PSUM->SBUF casts/copies run on DVE (Pool cannot touch PSUM); the
diagonal causal masks run on Pool so the DVE queue stays short.
"""

import numpy as np
import ml_dtypes

import concourse.bass as bass
import concourse.mybir as mybir
import concourse.tile as tile
from concourse import bacc
from concourse.bass_utils import run_bass_kernel_spmd
from concourse.masks import make_identity

F32 = mybir.dt.float32
BF16 = mybir.dt.bfloat16
AF = mybir.ActivationFunctionType

T = 2048  # sequence length
D = 1024  # embed dim
NH = 4  # heads per core
HD = 64  # head dim
DH = NH * HD  # 256, sharded d per core
TT = T // 128  # 16 t tiles
DT = D // 128  # 8 embed tiles
NSLAB = 4  # q slabs of 512
SCALE = 1.0 / np.sqrt(HD)

_NC_CACHE = None

def build():
    nc = bacc.Bacc(None, target_bir_lowering=False, debug=False)

    # host pre-layouts: partition-major so every DMA descriptor is one
    # contiguous 4KB run per partition
    xT_ext = nc.declare_dram_parameter("xT", [128, 8 * DT * 256], BF16, isOutput=False)
    wq = nc.declare_dram_parameter("Wq", [128, DT * DH], BF16, isOutput=False)
    wk = nc.declare_dram_parameter("Wk", [128, DT * DH], BF16, isOutput=False)
    wv = nc.declare_dram_parameter("Wv", [128, DT * DH], BF16, isOutput=False)
    wo = nc.declare_dram_parameter("Wo", [128, 2 * D], BF16, isOutput=False)
    # partials exported bf16: host sums 4 per-head-group partials in f32,
    # so the added quantization (~0.1% rel) is negligible and the output
    # DMA traffic halves
    out = nc.declare_dram_parameter("out", [T, D], BF16, isOutput=True)

    with tile.TileContext(nc) as tc:
        with (
            tc.tile_pool(name="persist", bufs=1) as persist,
            tc.tile_pool(name="pt", bufs=2) as pt_pool,
            tc.tile_pool(name="opev", bufs=3) as opev_pool,
            tc.tile_pool(name="avstg", bufs=2) as avstg_pool,
            tc.tile_pool(name="recip", bufs=4) as recip_pool,
            tc.tile_pool(name="ps_st", bufs=3, space="PSUM") as ps_st,
            tc.tile_pool(name="ps_av", bufs=2, space="PSUM") as ps_av,
        ):
            def P(shape, dtype, name):
                return persist.tile(shape, dtype, name=name, tag=name)

            ident_b = P([128, 128], BF16, "ident_b")
            make_identity(nc, ident_b)

            wq_bf = P([128, DT * DH], BF16, "wq_bf")
            wk_bf = P([128, DT * DH], BF16, "wk_bf")
            wv_bf = P([128, DT * DH], BF16, "wv_bf")
            wo_bf = P([128, 2 * D], BF16, "wo_bf")
            xT = P([128, DT * T], BF16, "xT")
            QT = P([128, 2 * T], BF16, "QT")
            KT = P([128, 2 * T], BF16, "KT")
            vbuf = P([128, TT * NH * 65], BF16, "vbuf")
            attn = P([128, TT * DH], BF16, "attn")
            attnT = P([128, 2 * T], BF16, "attnT")

            # ---- input DMAs: host layout is partition-major, so each
            # dma_start is 128 descriptors of one contiguous 4KB run.
            # xT arrives in 256-col stripes (all 8 dt chunks per stripe)
            # so V tiles and QK chunks unlock incrementally. ----
            SW = 8 * 256  # sbuf cols per stripe (8 dt x 256 t)

            def x_stripe(eng, j):
                eng.dma_start(
                    out=xT[:, j * SW : (j + 1) * SW],
                    in_=xT_ext[:, j * SW : (j + 1) * SW],
                )

            nc.scalar.dma_start(out=wv_bf[:], in_=wv[:])
            x_stripe(nc.sync, 0)
            nc.scalar.dma_start(out=wq_bf[:], in_=wq[:])
            x_stripe(nc.sync, 1)
            nc.scalar.dma_start(out=wk_bf[:], in_=wk[:])
            x_stripe(nc.sync, 2)
            nc.scalar.dma_start(out=wo_bf[:], in_=wo[:])
            for j in range(3, 8):
                x_stripe(nc.sync, j)

            vb3 = vbuf.rearrange("p (t c) -> p t c", c=65)
            nc.gpsimd.memset(vb3[:, :, 64:65], 1.0)

            # causal mask for the 128-wide diagonal blocks: keep col >= row
            tri = P([128, 128], BF16, "tri")
            nc.gpsimd.memset(tri[:], 1.0)
            nc.gpsimd.affine_select(
                out=tri[:],
                in_=tri[:],
                pattern=[[1, 128]],
                compare_op=mybir.AluOpType.is_ge,
                fill=0.0,
                base=0,
                channel_multiplier=-1,
            )

            xT4 = xT.rearrange("p (j dt c) -> p j dt c", j=8, dt=DT)

            def qk_chunk(w_bf, outT, m, c):
                """One 512-col QK projection chunk (PE + DVE cast)."""
                def go():
                    ps = ps_st.tile([128, 512], F32, name="psst", tag="ps")
                    for dt_ in range(DT):
                        nc.tensor.matmul(
                            ps[:],
                            lhsT=w_bf[
                                :, dt_ * DH + m * 128 : dt_ * DH + (m + 1) * 128
                            ],
                            rhs=xT4[:, 2 * c : 2 * c + 2, dt_, :],
                            start=(dt_ == 0),
                            stop=(dt_ == DT - 1),
                        )
                    nc.vector.tensor_copy(
                        outT[:, m * T + c * 512 : m * T + (c + 1) * 512], ps[:]
                    )

                return go

            vb4 = vbuf.rearrange("p (n c) -> p n c", c=65)

            def v_chunk(tt):
                """One V-projection t-tile (PE + Pool cast)."""
                def go():
                    ps = ps_st.tile([128, 512], F32, name="psst", tag="ps")
                    j, o = tt // 2, (tt % 2) * 128
                    for dt_ in range(DT):
                        nc.tensor.matmul(
                            ps[:, 0:256],
                            lhsT=xT4[:, j, dt_, o : o + 128],
                            rhs=wv_bf[:, dt_ * DH : (dt_ + 1) * DH],
                            start=(dt_ == 0),
                            stop=(dt_ == DT - 1),
                        )
                    nc.vector.tensor_copy(
                        vb4[:, tt * NH : (tt + 1) * NH, 0:64],
                        ps[:, 0:256].rearrange("p (n c) -> p n c", n=NH),
                    )

                return go

            def pt_layout(s):
                """Compact per-pair PT layout: col base and q-offset per kt."""
                base, off, b = {}, {}, 0
                for kt in range(4 * (s + 1)):
                    j = kt - 4 * s
                    o = 128 * j if j > 0 else 0
                    base[kt], off[kt] = b, o
                    b += 512 - o
                return base, off

            def scores_chunks(s, h, pt):
                m, r0 = h // 2, (h % 2) * 64
                base, _ = pt_layout(s)

                def off_diag(kt):
                    def go():
                        ps = ps_st.tile([128, 1024], F32, name="psst", tag="ps")
                        for u in range(2):
                            nc.tensor.matmul(
                                ps[:, u * 512 : (u + 1) * 512],
                                lhsT=KT[
                                    r0 : r0 + 64,
                                    m * T + (kt + u) * 128 : m * T + (kt + u + 1) * 128,
                                ],
                                rhs=QT[
                                    r0 : r0 + 64,
                                    m * T + s * 512 : m * T + (s + 1) * 512,
                                ],
                                start=True,
                                stop=True,
                            )
                        nc.scalar.activation(
                            out=pt[:, base[kt] : base[kt] + 1024],
                            in_=ps[:],
                            func=AF.Exp,
                            scale=float(SCALE),
                        )

                    return go

                def diag2(j0):
                    # two diagonal tiles (j0, j0+1) packed into one psum/exp
                    widths = [512 - 128 * j0, 512 - 128 * (j0 + 1)]
                    wtot = sum(widths)

                    def go():
                        ps = ps_st.tile([128, 1024], F32, name="psst", tag="ps")
                        o = 0
                        for u, w in enumerate(widths):
                            j = j0 + u
                            kt = 4 * s + j
                            nc.tensor.matmul(
                                ps[:, o : o + w],
                                lhsT=KT[
                                    r0 : r0 + 64,
                                    m * T + kt * 128 : m * T + (kt + 1) * 128,
                                ],
                                rhs=QT[
                                    r0 : r0 + 64,
                                    m * T + s * 512 + 128 * j : m * T + (s + 1) * 512,
                                ],
                                start=True,
                                stop=True,
                            )
                            o += w
                        kt0 = 4 * s + j0
                        nc.scalar.activation(
                            out=pt[:, base[kt0] : base[kt0] + wtot],
                            in_=ps[:, 0:wtot],
                            func=AF.Exp,
                            scale=float(SCALE),
                        )
                        for u in range(2):
                            kt = 4 * s + j0 + u
                            nc.gpsimd.tensor_mul(
                                pt[:, base[kt] : base[kt] + 128],
                                pt[:, base[kt] : base[kt] + 128],
                                tri[:],
                            )

                    return go

                return [off_diag(2 * u) for u in range(2 * s)] + [diag2(0), diag2(2)]

            def av_ops(s, h, pt, eager=False):
                """V-stationary AV accumulation; batched transpose+norm.
                eager=True computes recip/norm per q-tile immediately so the
                drain epilogue can start while later tiles still transpose."""
                base, off = pt_layout(s)
                nk = 4 * (s + 1)
                stg = {}

                def av_go():
                    avb = ps_av.tile([128, 512], F32, name="psav", tag="psav")
                    for kt in range(nk):
                        o = off[kt]
                        nc.tensor.matmul(
                            avb[0:65, o:512],
                            lhsT=vb4[:, kt * NH + h, :],
                            rhs=pt[:, base[kt] : base[kt] + 512 - o],
                            start=(kt == 0),
                            stop=(kt == nk - 1),
                        )
                    st = avstg_pool.tile([65, 512], BF16, name="avst")
                    stg["st"] = st
                    nc.vector.tensor_copy(st[:], avb[0:65, :])

                pnst = {}

                def tr_go(qi):
                    def go():
                        st = stg["st"]
                        if qi == 0:
                            pnst["pn"] = ps_av.tile(
                                [128, 264], BF16, name="psn", tag="psav"
                            )
                            pnst["rc"] = recip_pool.tile(
                                [128, 4], F32, name="rc"
                            )
                        pn = pnst["pn"]
                        nc.tensor.transpose(
                            pn[:, qi * 66 : qi * 66 + 65],
                            st[:, qi * 128 : (qi + 1) * 128],
                            ident_b[0:65, 0:65],
                        )
                        rc = pnst["rc"]
                        pn66 = pn.rearrange("p (n c) -> p n c", c=66)
                        if eager:
                            nc.vector.reciprocal(
                                rc[:, qi : qi + 1], pn66[:, qi : qi + 1, 64]
                            )
                        elif qi == 3:
                            nc.vector.reciprocal(rc[:], pn66[:, :, 64])

                    return go

                def norm_go(qi):
                    def go():
                        qt = 4 * s + qi
                        pn, rc = pnst["pn"], pnst["rc"]
                        nc.vector.tensor_scalar_mul(
                            attn[:, qt * DH + h * 64 : qt * DH + (h + 1) * 64],
                            pn[:, qi * 66 : qi * 66 + 64],
                            rc[:, qi : qi + 1],
                        )

                    return go

                if eager:
                    ops = [av_go]
                    for qi in range(4):
                        ops += [tr_go(qi), norm_go(qi)]
                    return ops
                return (
                    [av_go]
                    + [tr_go(qi) for qi in range(4)]
                    + [norm_go(qi) for qi in range(4)]
                )

            at3 = attnT.rearrange("p (i t) -> p i t", i=2)

            def epilogue_ops(s):
                ops = []
                for qt in range(4 * s, 4 * (s + 1)):
                    def tr(qt=qt):
                        ps = ps_av.tile([128, 256], BF16, name="pstrb", tag="psav")
                        for i in range(2):
                            nc.tensor.transpose(
                                ps[:, i * 128 : (i + 1) * 128],
                                attn[:, qt * DH + i * 128 : qt * DH + (i + 1) * 128],
                                ident_b[:],
                            )
                        nc.vector.tensor_copy(
                            at3[:, :, qt * 128 : (qt + 1) * 128],
                            ps.rearrange("p (i c) -> p i c", i=2),
                        )

                    ops.append(tr)
                for tt in range(4 * s, 4 * (s + 1)):
                    def op_(tt=tt):
                        ps = ps_st.tile([128, 1024], F32, name="psst", tag="ps")
                        for i in range(2):
                            lhsT = attnT[:, i * T + tt * 128 : i * T + (tt + 1) * 128]
                            for ec in range(2):
                                nc.tensor.matmul(
                                    ps[:, ec * 512 : (ec + 1) * 512],
                                    lhsT=lhsT,
                                    rhs=wo_bf[
                                        :, i * D + ec * 512 : i * D + (ec + 1) * 512
                                    ],
                                    start=(i == 0),
                                    stop=(i == 1),
                                )
                        ev = opev_pool.tile([128, 1024], BF16, name="ev")
                        nc.vector.tensor_copy(ev[:], ps[:])
                        nc.sync.dma_start(
                            out=out[tt * 128 : (tt + 1) * 128, :], in_=ev[:]
                        )

                    ops.append(op_)
                return ops

            def interleave(a, b):
                if not a:
                    return list(b)
                if not b:
                    return list(a)
                res = []
                nb, na, bi = len(b), len(a), 0
                for i, op in enumerate(a):
                    res.append(op)
                    want = (i + 1) * nb // na
                    while bi < want:
                        res.append(b[bi])
                        bi += 1
                res.extend(b[bi:])
                return res

            # projection thunks: qk[(w, m, c)] and v[tt]
            qkt = {}
            for w_bf, outT, wn in ((wq_bf, QT, "q"), (wk_bf, KT, "k")):
                for m in range(2):
                    for c in range(4):
                        qkt[(wn, m, c)] = qk_chunk(w_bf, outT, m, c)
            vt = {tt: v_chunk(tt) for tt in range(TT)}

            # ---- minimal prologue: just enough for (slab 0, head 0) ----
            for op in (vt[0], vt[1], qkt[("q", 0, 0)], qkt[("k", 0, 0)]):
                op()

            # remaining projection work as PE filler, placed just before
            # its first consumer in the pair stream
            fill_at = {
                0: [vt[2], vt[3], qkt[("q", 1, 0)], qkt[("k", 1, 0)]],
                1: [qkt[("q", 0, 1)], qkt[("k", 0, 1)]],
                2: [vt[4], vt[5]],
                3: [vt[6], vt[7], qkt[("q", 1, 1)]],
                4: [qkt[("k", 1, 1)], qkt[("q", 0, 2)]],
                5: [qkt[("k", 0, 2)], vt[8]],
                6: [vt[9], vt[10], vt[11]],
                7: [qkt[("q", 1, 2)], qkt[("k", 1, 2)]],
                8: [qkt[("q", 0, 3)], qkt[("k", 0, 3)]],
                9: [qkt[("q", 1, 3)], qkt[("k", 1, 3)]],
                10: [vt[12]],
                11: [vt[13]],
                12: [vt[14], vt[15]],
            }
            # slab epilogues: deferred into the ACT-heavy slab-3 stretch
            epi_at = {13: 1, 14: 2, 15: 3}

            pairs = (
                [(0, 0), (0, 1), (0, 2)]
                + [(1, h) for h in range(4)]
                + [(2, h) for h in range(4)]
                + [(3, h) for h in range(4)]
                + [(0, 3)]
            )
            pts = {}
            prev = None
            for idx in range(len(pairs) + 1):
                sc = []
                if idx < len(pairs):
                    s, h = pairs[idx]
                    pts[idx] = pt_pool.tile([128, TT * 512], BF16, name="pt")
                    sc = scores_chunks(s, h, pts[idx])
                av = []
                if prev is not None:
                    ps_, ph_ = pairs[prev]
                    av = av_ops(ps_, ph_, pts[prev], eager=(idx == len(pairs)))
                fill = fill_at.get(idx, [])
                epi = epilogue_ops(epi_at[idx]) if idx in epi_at else []
                if idx == len(pairs):
                    # drain: eager AV of (0,3) interleaved with the slab-0
                    # epilogue so PE never waits on the norm chain
                    e0 = epilogue_ops(0)
                    order = (
                        av[0:5]
                        + [e0[0], av[5], av[6], e0[1], av[7], av[8]]
                        + e0[2:]
                    )
                    for op in order:
                        op()
                else:
                    for op in interleave(sc, fill + av + epi):
                        op()
                prev = idx

    nc.compile()
    return nc


def _get_nc():
    global _NC_CACHE
    if _NC_CACHE is None:
        _NC_CACHE = build()
    return _NC_CACHE


def _pmajor_w(w):
    # [1024, 256] -> [128, 8*256]: whost[p, dt*256+c] = w[dt*128+p, c]
    return np.ascontiguousarray(
        w.reshape(DT, 128, DH).transpose(1, 0, 2).reshape(128, DT * DH)
    )


def make_in_maps(x, Wq, Wk, Wv, Wo):
    bf = ml_dtypes.bfloat16
    x = np.asarray(x, dtype=np.float32)
    WqT = np.asarray(Wq, dtype=np.float32).astype(bf)
    WkT = np.asarray(Wk, dtype=np.float32).astype(bf)
    WvT = np.asarray(Wv, dtype=np.float32).astype(bf)
    WoT = np.asarray(Wo, dtype=np.float32).astype(bf)
    # xT host layout: [p, j, dt, c] (j = 256-col stripe, dt = 128-row chunk)
    xTb = []
    for b in range(2):
        xt = x[b].T.astype(bf)  # [1024, 2048]
        xt = xt.reshape(DT, 128, 8, 256).transpose(1, 2, 0, 3)
        xTb.append(np.ascontiguousarray(xt.reshape(128, 8 * DT * 256)))
    in_maps = []
    for core in range(8):
        b, g = core // 4, core % 4
        sl = slice(g * DH, (g + 1) * DH)
        wo_c = WoT[sl, :]  # [256, 1024]
        wo_p = np.ascontiguousarray(
            wo_c.reshape(2, 128, D).transpose(1, 0, 2).reshape(128, 2 * D)
        )
        in_maps.append(
            {
                "xT": xTb[b],
                "Wq": _pmajor_w(WqT[:, sl]),
                "Wk": _pmajor_w(WkT[:, sl]),
                "Wv": _pmajor_w(WvT[:, sl]),
                "Wo": wo_p,
            }
        )
    return in_maps


def unshard(results):
    out = np.zeros((2, T, D), np.float32)
    for b in range(2):
        for g in range(4):
            out[b] += results[4 * b + g]["out"].astype(np.float32)
    return out


def kernel(x, Wq, Wk, Wv, Wo):
    nc = _get_nc()
    in_maps = make_in_maps(x, Wq, Wk, Wv, Wo)
    res = run_bass_kernel_spmd(nc, in_maps, core_ids=list(range(8)))
    return unshard(res.results)
